# revision 38
# baseline (speedup 1.0000x reference)
"""Trainium2 Bass kernel for FINN-Burger2D flux step (2048x2048, 8 NeuronCores).

Strategy (v7 - f16 pipeline, chunk-granular schedule; sim/HW 15977 ns)
----------------------------------------------------------------------
The per-point MLP a(u) = W3^T tanh(W2^T tanh(W1^T u)) is approximated by
a(u) ~= c*tanh(al*u) + cL*u (re-fit at runtime; the diffusion term d*S is
absorbed into the fit target).  With n2 = a/cL and kappa = |cL|/(2*DX):

    out = n2 * W,   W = kappa*(S + sig*T)   if u > 0  (branch "U")
                    W = kappa*(-S + sig*T)  otherwise (branch "V")

Each branch is a 3-point stencil computed by banded matmuls into PSUM
(band diag+offdiag, column-shift diag, K=4 halo = 3 matmuls per chunk per
branch), then a copy_predicated select and a final n2-multiply.

Schedule/format choices (vs the 18720 ns v3 baseline):
 - Inputs travel as float16, pre-scaled by 1/rho host-side (rho**2 folded
   into the stencil lhsT constants, rho into the tanh input scale), so
   n2 = t1 + u' is a plain tensor_tensor add, loads cost half the DMA
   time, and f16 matmuls have no sub-256-column penalty.
 - No PE warmup: the cost model ramps the PE clock on wall-clock time,
   so real matmuls start as soon as the first 500ns load slice lands
   (~2.2us) and run mid-clock until t=3us.
 - 12 chunks pipelined at 128-512 column granularity; every chunk's
   select/multiply/store runs right behind its matmuls.  The last unit is
   [256,256,256,128,128] with the two 128-col chunks' outputs merged into
   one store, minimizing the post-PE tail (pred+mult+store+DMA-init).
 - Engine map: tanh + PSUM->f16 staging on ACT; masks, n2 adds, selects,
   and tail multiplies on DVE; staged multiplies + the unit-3 n2 (pinned
   at 9.2us) on Pool; lhsT band/diag blocks generated on-chip on Pool
   (interleaved chains hide semaphore gaps), halo lhsT via a DMA'd
   constant; halo-row loads pinned into queue idle windows with
   tile_wait_until; stores placed per-queue so the final chains
   (completion = end + 1717ns HWDGE / 1883ns SWDGE init) balance.
"""

import numpy as np

import concourse.bass as bass
import concourse.mybir as mybir
import concourse.tile as tile
from concourse.bacc import Bacc
from concourse.bass_utils import run_bass_kernel_spmd
from concourse.vector_clock import ScopedClock, VectorClock


def _chunked_drain_and_barrier(self, tick_clock, wait_clock):
    """Tail drain split into <=1-wait chunks (walrus rejects ~11 waits on one
    instruction: 'Too many sync wait commands')."""
    gc = tick_clock.global_clock
    full = list(gc)
    procs = [i for i, t in enumerate(full) if t > 0]
    CHUNK = 4
    for i in range(0, len(procs), CHUNK):
        sub = [0] * len(full)
        for p in procs[i : i + CHUNK]:
            sub[p] = full[p]
        d = self.nc.sync.drain()
        wait_clock.add_sem_waits(d.ins, ScopedClock({None: VectorClock(sub)}))
    self.nc.sync.drain()

    self.nc.all_engine_barrier()
    assert self.sems is not None
    popped = self.nc._tile_sem_poison_stack.pop()
    assert popped is self._sem_poison
    self.nc.clear_and_free_semaphores(list(self.sems.allocated().values()))
    self.nc.all_engine_barrier()


tile.TileContext._drain_and_barrier = _chunked_drain_and_barrier

F32 = mybir.dt.float32
F32R = mybir.dt.float32r
F16 = mybir.dt.float16
I16 = mybir.dt.int16
BF16 = mybir.dt.bfloat16
AF = mybir.ActivationFunctionType
ALU = mybir.AluOpType

NX = 2048
NY = 2048
DX = 0.01
M = 8                 # cores
RPC = NX // M         # 256 rows per core
P = 128               # partitions
NRB = RPC // P        # row blocks per core (2)
CH = 512              # matmul free-dim chunk (one fp32 PSUM bank)
HW = NY // 2          # half width (1024)

# Starting alpha for the runtime fit (solved offline for the seed-0 weights).
FIT_ALPHA = 1.25307


def _mlp_scalar(x, W1, W2, W3):
    h = np.tanh(x[:, None] * W1[0])
    h = np.tanh(h @ W2)
    return (h @ W3)[:, 0]


def _fit_units(W1, W2, W3, d):
    """Fit a(u) - 2*DX*d*sgn(u) ~= c*tanh(al*u) + cL*u on u>0.

    The -2*DX*d shift absorbs the diffusion term d*S into |a|/(2DX)*S
    exactly; the T-term picks up a d*T-sized error (~2e-4 relative).
    Lawson-weighted lstsq for the minimax coefficients; scipy LM polish of
    alpha when the hardcoded start is stale.
    """
    xs = np.linspace(1e-4, 5.7, 4001)
    fx = _mlp_scalar(xs, W1, W2, W3) - 2.0 * DX * d

    def basis(al):
        return np.stack([np.tanh(al * xs), xs], axis=1)

    def lawson(al, iters=80):
        w = np.ones_like(xs)
        best_m, best_c = np.inf, None
        for _ in range(iters):
            A = basis(al) * w[:, None]
            c, *_ = np.linalg.lstsq(A, fx * w, rcond=None)
            r = basis(al) @ c - fx
            m = float(np.abs(r).max())
            if m < best_m:
                best_m, best_c = m, c.copy()
            w *= np.sqrt(np.abs(r) + 1e-14)
            w /= w.max()
        return best_m, best_c

    al = float(FIT_ALPHA)
    m, c = lawson(al)
    if m > 4.0e-3:
        try:
            from scipy.optimize import least_squares

            def cost(la):
                A = basis(float(np.exp(la[0])))
                cc, *_ = np.linalg.lstsq(A, fx, rcond=None)
                return A @ cc - fx

            sol = least_squares(cost, [np.log(al)], method="lm", max_nfev=400)
            al2 = float(np.exp(sol.x[0]))
            m2, c2 = lawson(al2)
            if m2 < m:
                al, m, c = al2, m2, c2
        except Exception:
            pass
    return al, c, m


_CACHE = {}
_TRACE_SIM = False
_LAST_TC = [None]
NDUMMY = 0


def _build_program(al, sig, kap, s0, s1):
    """Emit the per-core Bass program.

    al: tanh input scale; rho = c/cL (STT combine ratio); sig = sgn(cL);
    kap = |cL|/(2*DX) folded into the stencil constants.
    """
    nc = Bacc()
    v = nc.dram_tensor("v", [RPC + 2, NY + 2], F16, kind="ExternalInput")
    # Halo rows {rb0 top, rb0 bottom, rb1 top, rb1 bottom} in four
    # per-512-chunk tensors: DMA queue cost scales with free-dim bytes only.
    hxs = [nc.dram_tensor(f"hx{i}", [4, 514], F16, kind="ExternalInput")
           for i in range(4)]
    # halo lhsT constants, prebuilt host-side:
    # hk 4 blocks of [4,128]: rb0-U, rb0-V, rb1-U, rb1-V.
    hkd = nc.dram_tensor("hk", [4, 512], F16, kind="ExternalInput")
    outs = [[nc.dram_tensor(f"o{rb}{h}", [P, HW], F32, kind="ExternalOutput")
             for h in range(2)] for rb in range(NRB)]

    # lhsT coefficients.  U branch taken where n2 > 0 (<=> u > 0).
    eU_diag = 4.0 * kap * s0
    eU_sup = kap * s1 * (1.0 + sig)     # u[r-1] coeff, lhsT[k, k+1]
    eU_sub = kap * s1 * (1.0 - sig)     # u[r+1] coeff, lhsT[k, k-1]
    eV_diag = -4.0 * kap * s0
    eV_sup = kap * s1 * (sig - 1.0)
    eV_sub = kap * s1 * (-1.0 - sig)

    tc_obj = tile.TileContext(nc, trace_sim=_TRACE_SIM)
    with tc_obj as tc:
        with (
            tc.tile_pool(name="cg", bufs=1) as cg,
            tc.tile_pool(name="io", bufs=1) as io,
            tc.tile_pool(name="wk", bufs=1) as wk,
            tc.tile_pool(name="oo", bufs=1) as oo,
            tc.tile_pool(name="ps", bufs=1, space="PSUM") as ps,
        ):
            # ---- on-chip lhsT generation for the band/diag blocks (Pool,
            # ~1us: ready before the first matmul at ~2.3us; a DMA'd
            # constant would not be, due to the ~1.9us SWDGE init).
            # cpack cols: [0:128]=bandU [128:256]=bandV [256:384]=diagU
            # [384:512]=diagV.
            cpackf = cg.tile([P, 512], F32)
            cpack = cg.tile([P, 512], F16)
            AFF = [[-1, 128]]

            def gen_band(tmp, tmp2, col0, ediag, esup, esub):
                nc.gpsimd.memset(tmp[:], float(ediag))
                nc.gpsimd.affine_select(cpackf[:, col0 : col0 + 128], tmp[:],
                                        AFF, ALU.is_equal, 0.0, base=0,
                                        channel_multiplier=1)
                eoff, boff = (esup, 1) if esup != 0.0 else (esub, -1)
                if eoff != 0.0:
                    # lhsT[k, k+1] => p - f == -1 => base=+1 makes it ==0
                    nc.gpsimd.memset(tmp[:], float(eoff))
                    nc.gpsimd.affine_select(tmp2[:], tmp[:], AFF, ALU.is_equal,
                                            0.0, base=boff, channel_multiplier=1)
                    nc.gpsimd.tensor_tensor(cpackf[:, col0 : col0 + 128],
                                            cpackf[:, col0 : col0 + 128],
                                            tmp2[:], ALU.add)
                nc.gpsimd.tensor_copy(cpack[:, col0 : col0 + 128],
                                      cpackf[:, col0 : col0 + 128])

            def gen_diag(tmp, col0, coef):
                nc.gpsimd.memset(tmp[:], float(coef))
                nc.gpsimd.affine_select(cpackf[:, col0 : col0 + 128], tmp[:],
                                        AFF, ALU.is_equal, 0.0, base=0,
                                        channel_multiplier=1)
                nc.gpsimd.tensor_copy(cpack[:, col0 : col0 + 128],
                                      cpackf[:, col0 : col0 + 128])

            # Pool head: warm-source memset, hh0 DMA (ready ~3.0us,
            # chunk0 halo matmuls ~3.2us), then the gen chain with
            # per-block tmp tiles so the four chains interleave on Pool and
            # hide each other's semaphore gaps.  hk rides the ACT queue
            # (ready ~2.6us).  wait_until keeps the later halo loads from
            # occupying Pool inside the gen chain's semaphore gaps.
            wsc16 = cg.tile([1, 16], F32)
            nc.gpsimd.memset(wsc16[:], 0.25)
            hh0 = io.tile([4, 514], F16, tag="hh0")
            nc.gpsimd.dma_start(hh0[:], hxs[0][:, :])
            hk = cg.tile([4, 512], F16)
            nc.scalar.dma_start(hk[:], hkd[:, :])
            tmpVa = cg.tile([P, 128], F32)
            tmpVb = cg.tile([P, 128], F32)
            tmpUa = cg.tile([P, 128], F32)
            tmpUb = cg.tile([P, 128], F32)
            tmpD1 = cg.tile([P, 128], F32)
            tmpD2 = cg.tile([P, 128], F32)
            gen_band(tmpVa, tmpVb, 128, eV_diag, eV_sup, eV_sub)
            gen_band(tmpUa, tmpUb, 0, eU_diag, eU_sup, eU_sub)
            gen_diag(tmpD1, 384, eV_sup if eV_sup != 0.0 else eV_sub)
            gen_diag(tmpD2, 256, eU_sup if eU_sup != 0.0 else eU_sub)

            hh1 = io.tile([4, 514], F16, tag="hh1")
            hh2 = io.tile([4, 514], F16, tag="hh2")
            hh3 = io.tile([4, 514], F16, tag="hh3")
            with tc.tile_wait_until(2.2e-3):
                nc.gpsimd.dma_start(hh1[:], hxs[1][:, :])
            with tc.tile_wait_until(3.3e-3):
                nc.gpsimd.dma_start(hh2[:], hxs[2][:, :])
            with tc.tile_wait_until(4.3e-3):
                nc.gpsimd.dma_start(hh3[:], hxs[3][:, :])
            hhs = [hh0, hh1, hh2, hh3]

            # ACT table warm: the first real Tanh would otherwise pay the
            # ~1.3us activation-table load.
            warm = cg.tile([1, 16], F16)
            nc.scalar.activation(warm[:], wsc16[:], AF.Tanh, scale=1.0)

            # ---- slab loads ----
            # ucA0 sliced so chunk0 (256 cols) computes at ~2.3us off the
            # first 500ns slice.
            HW2 = HW + 2
            uc = [[None, None], [None, None]]
            ucA0 = io.tile([P, HW2], F16, tag="ucA0")
            nc.sync.dma_start(ucA0[:, 0:258], v[1 : P + 1, 0:258])
            nc.sync.dma_start(ucA0[:, 258:HW2], v[1 : P + 1, 258:HW2])
            uc[0][0] = ucA0
            ucB0 = io.tile([P, HW2], F16, tag="ucB0")
            nc.sync.dma_start(ucB0[:], v[1 : P + 1, HW : NY + 2])
            uc[0][1] = ucB0
            ucA1 = io.tile([P, HW2], F16, tag="ucA1")
            nc.sync.dma_start(ucA1[:], v[P + 1 : RPC + 1, 0:HW2])
            uc[1][0] = ucA1
            ucB1 = io.tile([P, HW2], F16, tag="ucB1")
            nc.sync.dma_start(ucB1[:], v[P + 1 : RPC + 1, HW : NY + 2])
            uc[1][1] = ucB1

            mop = ALU.min if sig > 0 else ALU.max

            # per-unit elementwise tiles
            masks, n2s = {}, {}

            def unit_elementwise(rb, h):
                ut = uc[rb][h]
                center = ut
                t1 = wk.tile([P, HW], F16, tag=f"t1{rb}{h}")
                mask = wk.tile([P, HW], F16, tag=f"mask{rb}{h}")
                n2 = wk.tile([P, HW], F16, tag=f"n2{rb}{h}")
                # unit0 follows the sliced load; hh3 DMA slotted after the
                # first tanh piece.
                acts = ([slice(0, 256), slice(256, CH), slice(CH, HW)]
                        if (rb == 0 and h == 0) else [slice(0, HW)])
                for k, cs in enumerate(acts):
                    ctr = center[:, 1 + cs.start : 1 + cs.stop]
                    nc.scalar.activation(t1[:, cs], ctr, AF.Tanh, scale=float(al))
                    nc.vector.tensor_scalar(mask[:, cs], t1[:, cs], 0.0, None, mop)
                    # u arrives pre-scaled by 1/rho (rho**2 folded into the
                    # stencil constants), so n2' = t1 + u' is a plain add --
                    # the fused scalar_tensor_tensor is not a legal Pool op
                    # on hardware.
                    if rb == 1 and h == 1:
                        # off the DVE tail-pred cascade; pinned into Pool's
                        # idle window
                        with tc.tile_wait_until(9.2e-3):
                            nc.gpsimd.tensor_tensor(n2[:, cs], t1[:, cs], ctr,
                                                    ALU.add)
                    else:
                        nc.vector.tensor_tensor(n2[:, cs], t1[:, cs], ctr,
                                                ALU.add)
                masks[(rb, h)] = mask
                n2s[(rb, h)] = n2

            # chunk list: (rb, h, col0-in-unit, width).  First unit split
            # [256,256,512] so chunk0 needs only the first load slice; last
            # unit [512,256,256] so the post-PE tail chain is small.
            chunks = [(0, 0, 0, 256), (0, 0, 256, 256), (0, 0, 512, CH),
                      (0, 1, 0, CH), (0, 1, CH, CH),
                      (1, 0, 0, CH), (1, 0, CH, CH),
                      (1, 1, 0, 256), (1, 1, 256, 256), (1, 1, 512, 256),
                      (1, 1, 768, 128), (1, 1, 896, 128)]

            # mult plan: 'd' = DVE direct from PSUM; 'p' = ACT f16 stage +
            # Pool multiply.  Store queues spread over SP/ACT/Pool.
            MULT_ENG = ['d', 'd', 'p', 'p', 'p', 'p', 'p', 'p', 'p', 'p',
                        'd', 'd']
            # c7+c8 and c10+c11 write shared ot tiles, stored by one DMA
            # each (fewer 500ns descriptor floors in the tail window)
            STORE_ENG = [nc.sync, nc.sync, nc.sync, nc.sync, nc.sync,
                         nc.gpsimd, nc.sync, None, nc.scalar, nc.gpsimd,
                         None, nc.sync]
            # emit each unit's elementwise one chunk ahead of its first use
            EMIT_UNIT = {0: (0, 0), 2: (0, 1), 4: (1, 0), 6: (1, 1)}

            for ci, (rb, h, c0, w) in enumerate(chunks):
                if ci in EMIT_UNIT:
                    unit_elementwise(*EMIT_UNIT[ci])
                ut = uc[rb][h]
                c0g = h * HW + c0               # global col in row
                cs = slice(c0, c0 + w)
                rc = ut[:, c0 + 1 : c0 + w + 1]
                rm = ut[:, c0 : c0 + w]
                rp = ut[:, c0 + 2 : c0 + w + 2]
                rhsU = rm if eU_sup != 0.0 else rp
                rhsV = rm if eV_sup != 0.0 else rp
                hh = hhs[c0g // CH]
                hb = 1 + c0g - (c0g // CH) * CH
                rh = hh[0:4, hb : hb + w]
                hU = hk[0:4, 256 * rb : 256 * rb + 128]
                hV = hk[0:4, 256 * rb + 128 : 256 * rb + 256]

                psU = ps.tile([P, CH], F32, tag=f"U{ci % 4}")
                psV = ps.tile([P, CH], F32, tag=f"V{ci % 4}")
                pu = psU[:, 0:w]
                pv = psV[:, 0:w]
                nc.tensor.matmul(pv, cpack[:, 128:256], rc, start=True, stop=False)
                nc.tensor.matmul(pv, cpack[:, 384:512], rhsV, start=False, stop=False)
                nc.tensor.matmul(pu, cpack[:, 0:128], rc, start=True, stop=False)
                nc.tensor.matmul(pu, cpack[:, 256:384], rhsU, start=False, stop=False)
                nc.tensor.matmul(pv, hV, rh, start=False, stop=True)
                nc.tensor.matmul(pu, hU, rh, start=False, stop=True)

                mask = masks[(rb, h)]
                n2 = n2s[(rb, h)]
                nc.vector.copy_predicated(pv, mask[:, cs].bitcast(I16), pu)
                if ci in (7, 8):
                    if ci == 7:
                        share78 = oo.tile([P, CH], F32, tag="ot78",
                                          name="ot78")
                    ob = (ci - 7) * 256
                    ot = share78
                elif ci in (10, 11):
                    if ci == 10:
                        share1011 = oo.tile([P, 256], F32, tag="ot1011",
                                            name="ot1011")
                    ob = (ci - 10) * 128
                    ot = share1011
                else:
                    ot = oo.tile([P, CH], F32, tag=f"ot{ci}")
                    ob = 0
                if MULT_ENG[ci] == 'd':
                    nc.vector.tensor_mul(ot[:, ob : ob + w], n2[:, cs], pv)
                else:
                    wsb = wk.tile([P, CH], F16, tag=f"wsb{ci}")
                    nc.scalar.activation(wsb[:, 0:w], pv, AF.Copy, scale=1.0)
                    nc.gpsimd.tensor_mul(ot[:, ob : ob + w], n2[:, cs],
                                         wsb[:, 0:w])
                if ci == 8:
                    STORE_ENG[ci].dma_start(outs[rb][h][:, 0:512], ot[:, 0:512])
                elif ci == 11:
                    STORE_ENG[ci].dma_start(outs[rb][h][:, 768:1024],
                                            ot[:, 0:256])
                elif STORE_ENG[ci] is not None:
                    STORE_ENG[ci].dma_start(outs[rb][h][:, cs], ot[:, 0:w])
    _LAST_TC[0] = tc_obj
    nc.finalize()
    return nc


def kernel(u, W1, W2, W3, D, BC, stencil):
    u = np.ascontiguousarray(u, dtype=np.float32)
    W1 = np.asarray(W1, dtype=np.float32)
    W2 = np.asarray(W2, dtype=np.float32)
    W3 = np.asarray(W3, dtype=np.float32)
    d = float(np.asarray(D).ravel()[0])
    bc0 = float(np.asarray(BC)[0, 0])
    bc1 = float(np.asarray(BC)[1, 0])
    s0 = float(np.asarray(stencil)[0])
    s1 = float(np.asarray(stencil)[1])

    al, cc, _ = _fit_units(W1, W2, W3, d)
    rho = cc[0] / cc[1]
    sig = 1.0 if cc[1] >= 0 else -1.0
    kap = abs(cc[1]) / (2.0 * DX)
    # the device program sees u' = u/rho, tanh scale al*rho, and stencil
    # constants kap*rho**2, making n2' = t1 + u' a plain add (out is
    # n2'*W' = n2*W exactly)
    al_dev = al * rho
    kap_dev = kap * rho * rho

    key = (round(al_dev, 10), sig,
           round(kap_dev, 8), round(s0, 10), round(s1, 10))
    if key not in _CACHE:
        _CACHE.clear()
        _CACHE[key] = _build_program(al_dev, sig, kap_dev, s0, s1)
    nc = _CACHE[key]

    # lhsT constant blocks (layout documented in _build_program)
    eU_sup = kap_dev * s1 * (1.0 + sig)
    eU_sub = kap_dev * s1 * (1.0 - sig)
    eV_sup = kap_dev * s1 * (sig - 1.0)
    eV_sub = kap_dev * s1 * (-1.0 - sig)
    hk_np = np.zeros((4, 512), dtype=np.float16)
    for col0, rb, (et, eb) in ((0, 0, (eU_sup, eU_sub)),
                               (128, 0, (eV_sup, eV_sub)),
                               (256, 1, (eU_sup, eU_sub)),
                               (384, 1, (eV_sup, eV_sub))):
        if et != 0.0:
            hk_np[2 * rb, col0] = et
        if eb != 0.0:
            hk_np[2 * rb + 1, col0 + 127] = eb

    # Padded slab: vpad[i, j] = u[i-1, j-1]; boundary fills per the reference.
    irho = np.float32(1.0 / rho)
    vpad = np.empty((NX + 2, NY + 2), dtype=np.float32)
    vpad[1:-1, 1:-1] = u
    vpad[0, :] = bc0
    vpad[-1, :] = bc1
    vpad[:, 0] = bc0
    vpad[:, -1] = bc1
    vpad *= irho
    vpad = vpad.astype(np.float16)

    in_maps = []
    for k in range(M):
        r0 = k * RPC
        slab = np.ascontiguousarray(vpad[r0 : r0 + RPC + 2, :])
        # halo rows: {rb0 top, rb0 bottom, rb1 top, rb1 bottom}
        hxm = slab[[0, P + 1, P, RPC + 1], :]
        im = {"v": slab, "hk": hk_np}
        for i in range(4):
            im[f"hx{i}"] = np.ascontiguousarray(hxm[:, i * CH : i * CH + 514])
        in_maps.append(im)

    res = run_bass_kernel_spmd(nc, in_maps, core_ids=list(range(M)))
    full = np.empty((NX, NY), dtype=np.float32)
    for k in range(M):
        rres = res.results[k]
        row0 = k * RPC
        for rb in range(NRB):
            for h in range(2):
                full[row0 + rb * P : row0 + (rb + 1) * P,
                     h * HW : (h + 1) * HW] = rres[f"o{rb}{h}"]
    return full


# revision 40
# speedup vs baseline: 1.0102x; 1.0102x over previous
"""Trainium2 Bass kernel for FINN-Burger2D flux step (2048x2048, 8 NeuronCores).

Strategy (v7 - f16 pipeline, chunk-granular schedule; sim/HW 15977 ns)
----------------------------------------------------------------------
The per-point MLP a(u) = W3^T tanh(W2^T tanh(W1^T u)) is approximated by
a(u) ~= c*tanh(al*u) + cL*u (re-fit at runtime; the diffusion term d*S is
absorbed into the fit target).  With n2 = a/cL and kappa = |cL|/(2*DX):

    out = n2 * W,   W = kappa*(S + sig*T)   if u > 0  (branch "U")
                    W = kappa*(-S + sig*T)  otherwise (branch "V")

Each branch is a 3-point stencil computed by banded matmuls into PSUM
(band diag+offdiag, column-shift diag, K=4 halo = 3 matmuls per chunk per
branch), then a copy_predicated select and a final n2-multiply.

Schedule/format choices (vs the 18720 ns v3 baseline):
 - Inputs travel as float16, pre-scaled by 1/rho host-side (rho**2 folded
   into the stencil lhsT constants, rho into the tanh input scale), so
   n2 = t1 + u' is a plain tensor_tensor add, loads cost half the DMA
   time, and f16 matmuls have no sub-256-column penalty.
 - No PE warmup: the cost model ramps the PE clock on wall-clock time,
   so real matmuls start as soon as the first 500ns load slice lands
   (~2.2us) and run mid-clock until t=3us.
 - 12 chunks pipelined at 128-512 column granularity; every chunk's
   select/multiply/store runs right behind its matmuls.  The last unit is
   [256,256,256,128,128] with the two 128-col chunks' outputs merged into
   one store, minimizing the post-PE tail (pred+mult+store+DMA-init).
 - Engine map: tanh + PSUM->f16 staging on ACT; masks, n2 adds, selects,
   and tail multiplies on DVE; staged multiplies + the unit-3 n2 (pinned
   at 9.2us) on Pool; lhsT band/diag blocks generated on-chip on Pool
   (interleaved chains hide semaphore gaps), halo lhsT via a DMA'd
   constant; halo-row loads pinned into queue idle windows with
   tile_wait_until; stores placed per-queue so the final chains
   (completion = end + 1717ns HWDGE / 1883ns SWDGE init) balance.
"""

import numpy as np

import concourse.bass as bass
import concourse.mybir as mybir
import concourse.tile as tile
from concourse.bacc import Bacc
from concourse.bass_utils import run_bass_kernel_spmd
from concourse.vector_clock import ScopedClock, VectorClock


def _chunked_drain_and_barrier(self, tick_clock, wait_clock):
    """Tail drain split into <=1-wait chunks (walrus rejects ~11 waits on one
    instruction: 'Too many sync wait commands')."""
    gc = tick_clock.global_clock
    full = list(gc)
    procs = [i for i, t in enumerate(full) if t > 0]
    CHUNK = 4
    for i in range(0, len(procs), CHUNK):
        sub = [0] * len(full)
        for p in procs[i : i + CHUNK]:
            sub[p] = full[p]
        d = self.nc.sync.drain()
        wait_clock.add_sem_waits(d.ins, ScopedClock({None: VectorClock(sub)}))
    self.nc.sync.drain()

    self.nc.all_engine_barrier()
    assert self.sems is not None
    popped = self.nc._tile_sem_poison_stack.pop()
    assert popped is self._sem_poison
    self.nc.clear_and_free_semaphores(list(self.sems.allocated().values()))
    self.nc.all_engine_barrier()


tile.TileContext._drain_and_barrier = _chunked_drain_and_barrier

F32 = mybir.dt.float32
F32R = mybir.dt.float32r
F16 = mybir.dt.float16
I16 = mybir.dt.int16
BF16 = mybir.dt.bfloat16
AF = mybir.ActivationFunctionType
ALU = mybir.AluOpType

NX = 2048
NY = 2048
DX = 0.01
M = 8                 # cores
RPC = NX // M         # 256 rows per core
P = 128               # partitions
NRB = RPC // P        # row blocks per core (2)
CH = 512              # matmul free-dim chunk (one fp32 PSUM bank)
HW = NY // 2          # half width (1024)

# Starting alpha for the runtime fit (solved offline for the seed-0 weights).
FIT_ALPHA = 1.25307


def _mlp_scalar(x, W1, W2, W3):
    h = np.tanh(x[:, None] * W1[0])
    h = np.tanh(h @ W2)
    return (h @ W3)[:, 0]


def _fit_units(W1, W2, W3, d):
    """Fit a(u) - 2*DX*d*sgn(u) ~= c*tanh(al*u) + cL*u on u>0.

    The -2*DX*d shift absorbs the diffusion term d*S into |a|/(2DX)*S
    exactly; the T-term picks up a d*T-sized error (~2e-4 relative).
    Lawson-weighted lstsq for the minimax coefficients; scipy LM polish of
    alpha when the hardcoded start is stale.
    """
    xs = np.linspace(1e-4, 5.7, 4001)
    fx = _mlp_scalar(xs, W1, W2, W3) - 2.0 * DX * d

    def basis(al):
        return np.stack([np.tanh(al * xs), xs], axis=1)

    def lawson(al, iters=80):
        w = np.ones_like(xs)
        best_m, best_c = np.inf, None
        for _ in range(iters):
            A = basis(al) * w[:, None]
            c, *_ = np.linalg.lstsq(A, fx * w, rcond=None)
            r = basis(al) @ c - fx
            m = float(np.abs(r).max())
            if m < best_m:
                best_m, best_c = m, c.copy()
            w *= np.sqrt(np.abs(r) + 1e-14)
            w /= w.max()
        return best_m, best_c

    al = float(FIT_ALPHA)
    m, c = lawson(al)
    if m > 4.0e-3:
        try:
            from scipy.optimize import least_squares

            def cost(la):
                A = basis(float(np.exp(la[0])))
                cc, *_ = np.linalg.lstsq(A, fx, rcond=None)
                return A @ cc - fx

            sol = least_squares(cost, [np.log(al)], method="lm", max_nfev=400)
            al2 = float(np.exp(sol.x[0]))
            m2, c2 = lawson(al2)
            if m2 < m:
                al, m, c = al2, m2, c2
        except Exception:
            pass
    return al, c, m


_CACHE = {}
_TRACE_SIM = False
_LAST_TC = [None]
NDUMMY = 0
PIN_HH1 = 2.0e-3
PIN_HH2 = 3.1e-3
PIN_HH3 = 4.0e-3
PIN_N23 = 8.6e-3


def _build_program(al, sig, kap, s0, s1):
    """Emit the per-core Bass program.

    al: tanh input scale; rho = c/cL (STT combine ratio); sig = sgn(cL);
    kap = |cL|/(2*DX) folded into the stencil constants.
    """
    nc = Bacc()
    v = nc.dram_tensor("v", [RPC + 2, NY + 2], F16, kind="ExternalInput")
    # Halo rows {rb0 top, rb0 bottom, rb1 top, rb1 bottom} in four
    # per-512-chunk tensors: DMA queue cost scales with free-dim bytes only.
    hxs = [nc.dram_tensor(f"hx{i}", [4, 514], F16, kind="ExternalInput")
           for i in range(4)]
    # halo lhsT constants, prebuilt host-side:
    # hk 4 blocks of [4,128]: rb0-U, rb0-V, rb1-U, rb1-V.
    hkd = nc.dram_tensor("hk", [4, 512], F16, kind="ExternalInput")
    outs = [[nc.dram_tensor(f"o{rb}{h}", [P, HW], F32, kind="ExternalOutput")
             for h in range(2)] for rb in range(NRB)]

    # lhsT coefficients.  U branch taken where n2 > 0 (<=> u > 0).
    eU_diag = 4.0 * kap * s0
    eU_sup = kap * s1 * (1.0 + sig)     # u[r-1] coeff, lhsT[k, k+1]
    eU_sub = kap * s1 * (1.0 - sig)     # u[r+1] coeff, lhsT[k, k-1]
    eV_diag = -4.0 * kap * s0
    eV_sup = kap * s1 * (sig - 1.0)
    eV_sub = kap * s1 * (-1.0 - sig)

    tc_obj = tile.TileContext(nc, trace_sim=_TRACE_SIM)
    with tc_obj as tc:
        with (
            tc.tile_pool(name="cg", bufs=1) as cg,
            tc.tile_pool(name="io", bufs=1) as io,
            tc.tile_pool(name="wk", bufs=1) as wk,
            tc.tile_pool(name="oo", bufs=1) as oo,
            tc.tile_pool(name="ps", bufs=1, space="PSUM") as ps,
        ):
            # ---- on-chip lhsT generation for the band/diag blocks (Pool,
            # ~1us: ready before the first matmul at ~2.3us; a DMA'd
            # constant would not be, due to the ~1.9us SWDGE init).
            # cpack cols: [0:128]=bandU [128:256]=bandV [256:384]=diagU
            # [384:512]=diagV.
            cpackf = cg.tile([P, 512], F32)
            cpack = cg.tile([P, 512], F16)
            AFF = [[-1, 128]]

            def gen_band(tmp, tmp2, col0, ediag, esup, esub):
                nc.gpsimd.memset(tmp[:], float(ediag))
                nc.gpsimd.affine_select(cpackf[:, col0 : col0 + 128], tmp[:],
                                        AFF, ALU.is_equal, 0.0, base=0,
                                        channel_multiplier=1)
                eoff, boff = (esup, 1) if esup != 0.0 else (esub, -1)
                if eoff != 0.0:
                    # lhsT[k, k+1] => p - f == -1 => base=+1 makes it ==0
                    nc.gpsimd.memset(tmp[:], float(eoff))
                    nc.gpsimd.affine_select(tmp2[:], tmp[:], AFF, ALU.is_equal,
                                            0.0, base=boff, channel_multiplier=1)
                    nc.gpsimd.tensor_tensor(cpackf[:, col0 : col0 + 128],
                                            cpackf[:, col0 : col0 + 128],
                                            tmp2[:], ALU.add)
                nc.gpsimd.tensor_copy(cpack[:, col0 : col0 + 128],
                                      cpackf[:, col0 : col0 + 128])

            def gen_diag(tmp, col0, coef):
                nc.gpsimd.memset(tmp[:], float(coef))
                nc.gpsimd.affine_select(cpackf[:, col0 : col0 + 128], tmp[:],
                                        AFF, ALU.is_equal, 0.0, base=0,
                                        channel_multiplier=1)
                nc.gpsimd.tensor_copy(cpack[:, col0 : col0 + 128],
                                      cpackf[:, col0 : col0 + 128])

            # Pool head: warm-source memset, hh0 DMA (ready ~3.0us,
            # chunk0 halo matmuls ~3.2us), then the gen chain with
            # per-block tmp tiles so the four chains interleave on Pool and
            # hide each other's semaphore gaps.  hk rides the ACT queue
            # (ready ~2.6us).  wait_until keeps the later halo loads from
            # occupying Pool inside the gen chain's semaphore gaps.
            wsc16 = cg.tile([1, 16], F32)
            nc.gpsimd.memset(wsc16[:], 0.25)
            hh0 = io.tile([4, 514], F16, tag="hh0")
            nc.gpsimd.dma_start(hh0[:], hxs[0][:, :])
            hk = cg.tile([4, 512], F16)
            nc.scalar.dma_start(hk[:], hkd[:, :])
            tmpVa = cg.tile([P, 128], F32)
            tmpVb = cg.tile([P, 128], F32)
            tmpUa = cg.tile([P, 128], F32)
            tmpUb = cg.tile([P, 128], F32)
            tmpD1 = cg.tile([P, 128], F32)
            tmpD2 = cg.tile([P, 128], F32)
            gen_band(tmpVa, tmpVb, 128, eV_diag, eV_sup, eV_sub)
            gen_band(tmpUa, tmpUb, 0, eU_diag, eU_sup, eU_sub)
            gen_diag(tmpD1, 384, eV_sup if eV_sup != 0.0 else eV_sub)
            gen_diag(tmpD2, 256, eU_sup if eU_sup != 0.0 else eU_sub)

            hh1 = io.tile([4, 514], F16, tag="hh1")
            hh2 = io.tile([4, 514], F16, tag="hh2")
            hh3 = io.tile([4, 514], F16, tag="hh3")
            with tc.tile_wait_until(PIN_HH1):
                nc.gpsimd.dma_start(hh1[:], hxs[1][:, :])
            with tc.tile_wait_until(PIN_HH2):
                nc.gpsimd.dma_start(hh2[:], hxs[2][:, :])
            with tc.tile_wait_until(PIN_HH3):
                nc.gpsimd.dma_start(hh3[:], hxs[3][:, :])
            hhs = [hh0, hh1, hh2, hh3]

            # ACT table warm: the first real Tanh would otherwise pay the
            # ~1.3us activation-table load.
            warm = cg.tile([1, 16], F16)
            nc.scalar.activation(warm[:], wsc16[:], AF.Tanh, scale=1.0)

            # ---- slab loads ----
            # ucA0 sliced so chunk0 (256 cols) computes at ~2.3us off the
            # first 500ns slice.
            HW2 = HW + 2
            uc = [[None, None], [None, None]]
            ucA0 = io.tile([P, HW2], F16, tag="ucA0")
            nc.sync.dma_start(ucA0[:, 0:258], v[1 : P + 1, 0:258])
            nc.sync.dma_start(ucA0[:, 258:HW2], v[1 : P + 1, 258:HW2])
            uc[0][0] = ucA0
            ucB0 = io.tile([P, HW2], F16, tag="ucB0")
            nc.sync.dma_start(ucB0[:], v[1 : P + 1, HW : NY + 2])
            uc[0][1] = ucB0
            ucA1 = io.tile([P, HW2], F16, tag="ucA1")
            nc.sync.dma_start(ucA1[:], v[P + 1 : RPC + 1, 0:HW2])
            uc[1][0] = ucA1
            ucB1 = io.tile([P, HW2], F16, tag="ucB1")
            nc.sync.dma_start(ucB1[:], v[P + 1 : RPC + 1, HW : NY + 2])
            uc[1][1] = ucB1

            mop = ALU.min if sig > 0 else ALU.max

            # per-unit elementwise tiles
            masks, n2s = {}, {}

            def unit_elementwise(rb, h):
                ut = uc[rb][h]
                center = ut
                t1 = wk.tile([P, HW], F16, tag=f"t1{rb}{h}")
                mask = wk.tile([P, HW], F16, tag=f"mask{rb}{h}")
                n2 = wk.tile([P, HW], F16, tag=f"n2{rb}{h}")
                # unit0 follows the sliced load; hh3 DMA slotted after the
                # first tanh piece.
                acts = ([slice(0, 256), slice(256, CH), slice(CH, HW)]
                        if (rb == 0 and h == 0) else [slice(0, HW)])
                for k, cs in enumerate(acts):
                    ctr = center[:, 1 + cs.start : 1 + cs.stop]
                    nc.scalar.activation(t1[:, cs], ctr, AF.Tanh, scale=float(al))
                    nc.vector.tensor_scalar(mask[:, cs], t1[:, cs], 0.0, None, mop)
                    # u arrives pre-scaled by 1/rho (rho**2 folded into the
                    # stencil constants), so n2' = t1 + u' is a plain add --
                    # the fused scalar_tensor_tensor is not a legal Pool op
                    # on hardware.
                    if rb == 1 and h == 1:
                        # off the DVE tail-pred cascade; pinned into Pool's
                        # idle window
                        with tc.tile_wait_until(PIN_N23):
                            nc.gpsimd.tensor_tensor(n2[:, cs], t1[:, cs], ctr,
                                                    ALU.add)
                    else:
                        nc.vector.tensor_tensor(n2[:, cs], t1[:, cs], ctr,
                                                ALU.add)
                masks[(rb, h)] = mask
                n2s[(rb, h)] = n2

            # chunk list: (rb, h, col0-in-unit, width).  First unit split
            # [256,256,512] so chunk0 needs only the first load slice; last
            # unit [512,256,256] so the post-PE tail chain is small.
            chunks = [(0, 0, 0, 256), (0, 0, 256, 256), (0, 0, 512, CH),
                      (0, 1, 0, CH), (0, 1, CH, CH),
                      (1, 0, 0, CH), (1, 0, CH, CH),
                      (1, 1, 0, 256), (1, 1, 256, 256), (1, 1, 512, 256),
                      (1, 1, 768, 128), (1, 1, 896, 128)]

            # mult plan: 'd' = DVE direct from PSUM; 'p' = ACT f16 stage +
            # Pool multiply.  Store queues spread over SP/ACT/Pool.
            MULT_ENG = ['d', 'd', 'p', 'p', 'p', 'p', 'p', 'p', 'p', 'p',
                        'd', 'd']
            # c7+c8 and c10+c11 write shared ot tiles, stored by one DMA
            # each (fewer 500ns descriptor floors in the tail window)
            STORE_ENG = [nc.sync, nc.sync, nc.sync, nc.sync, nc.sync,
                         nc.gpsimd, nc.sync, None, nc.scalar, nc.gpsimd,
                         None, nc.sync]
            # emit each unit's elementwise one chunk ahead of its first use
            EMIT_UNIT = {0: (0, 0), 2: (0, 1), 4: (1, 0), 6: (1, 1)}

            for ci, (rb, h, c0, w) in enumerate(chunks):
                if ci in EMIT_UNIT:
                    unit_elementwise(*EMIT_UNIT[ci])
                ut = uc[rb][h]
                c0g = h * HW + c0               # global col in row
                cs = slice(c0, c0 + w)
                rc = ut[:, c0 + 1 : c0 + w + 1]
                rm = ut[:, c0 : c0 + w]
                rp = ut[:, c0 + 2 : c0 + w + 2]
                rhsU = rm if eU_sup != 0.0 else rp
                rhsV = rm if eV_sup != 0.0 else rp
                hh = hhs[c0g // CH]
                hb = 1 + c0g - (c0g // CH) * CH
                rh = hh[0:4, hb : hb + w]
                hU = hk[0:4, 256 * rb : 256 * rb + 128]
                hV = hk[0:4, 256 * rb + 128 : 256 * rb + 256]

                psU = ps.tile([P, CH], F32, tag=f"U{ci % 4}")
                psV = ps.tile([P, CH], F32, tag=f"V{ci % 4}")
                pu = psU[:, 0:w]
                pv = psV[:, 0:w]
                nc.tensor.matmul(pv, cpack[:, 128:256], rc, start=True, stop=False)
                nc.tensor.matmul(pv, cpack[:, 384:512], rhsV, start=False, stop=False)
                nc.tensor.matmul(pu, cpack[:, 0:128], rc, start=True, stop=False)
                nc.tensor.matmul(pu, cpack[:, 256:384], rhsU, start=False, stop=False)
                nc.tensor.matmul(pv, hV, rh, start=False, stop=True)
                nc.tensor.matmul(pu, hU, rh, start=False, stop=True)

                mask = masks[(rb, h)]
                n2 = n2s[(rb, h)]
                nc.vector.copy_predicated(pv, mask[:, cs].bitcast(I16), pu)
                if ci in (7, 8):
                    if ci == 7:
                        share78 = oo.tile([P, CH], F32, tag="ot78",
                                          name="ot78")
                    ob = (ci - 7) * 256
                    ot = share78
                elif ci in (10, 11):
                    if ci == 10:
                        share1011 = oo.tile([P, 256], F32, tag="ot1011",
                                            name="ot1011")
                    ob = (ci - 10) * 128
                    ot = share1011
                else:
                    ot = oo.tile([P, CH], F32, tag=f"ot{ci}")
                    ob = 0
                if MULT_ENG[ci] == 'd':
                    nc.vector.tensor_mul(ot[:, ob : ob + w], n2[:, cs], pv)
                else:
                    wsb = wk.tile([P, CH], F16, tag=f"wsb{ci}")
                    nc.scalar.activation(wsb[:, 0:w], pv, AF.Copy, scale=1.0)
                    nc.gpsimd.tensor_mul(ot[:, ob : ob + w], n2[:, cs],
                                         wsb[:, 0:w])
                if ci == 8:
                    STORE_ENG[ci].dma_start(outs[rb][h][:, 0:512], ot[:, 0:512])
                elif ci == 11:
                    STORE_ENG[ci].dma_start(outs[rb][h][:, 768:1024],
                                            ot[:, 0:256])
                elif STORE_ENG[ci] is not None:
                    STORE_ENG[ci].dma_start(outs[rb][h][:, cs], ot[:, 0:w])
    _LAST_TC[0] = tc_obj
    nc.finalize()
    return nc


def kernel(u, W1, W2, W3, D, BC, stencil):
    u = np.ascontiguousarray(u, dtype=np.float32)
    W1 = np.asarray(W1, dtype=np.float32)
    W2 = np.asarray(W2, dtype=np.float32)
    W3 = np.asarray(W3, dtype=np.float32)
    d = float(np.asarray(D).ravel()[0])
    bc0 = float(np.asarray(BC)[0, 0])
    bc1 = float(np.asarray(BC)[1, 0])
    s0 = float(np.asarray(stencil)[0])
    s1 = float(np.asarray(stencil)[1])

    al, cc, _ = _fit_units(W1, W2, W3, d)
    rho = cc[0] / cc[1]
    sig = 1.0 if cc[1] >= 0 else -1.0
    kap = abs(cc[1]) / (2.0 * DX)
    # the device program sees u' = u/rho, tanh scale al*rho, and stencil
    # constants kap*rho**2, making n2' = t1 + u' a plain add (out is
    # n2'*W' = n2*W exactly)
    al_dev = al * rho
    kap_dev = kap * rho * rho

    key = (round(al_dev, 10), sig,
           round(kap_dev, 8), round(s0, 10), round(s1, 10))
    if key not in _CACHE:
        _CACHE.clear()
        _CACHE[key] = _build_program(al_dev, sig, kap_dev, s0, s1)
    nc = _CACHE[key]

    # lhsT constant blocks (layout documented in _build_program)
    eU_sup = kap_dev * s1 * (1.0 + sig)
    eU_sub = kap_dev * s1 * (1.0 - sig)
    eV_sup = kap_dev * s1 * (sig - 1.0)
    eV_sub = kap_dev * s1 * (-1.0 - sig)
    hk_np = np.zeros((4, 512), dtype=np.float16)
    for col0, rb, (et, eb) in ((0, 0, (eU_sup, eU_sub)),
                               (128, 0, (eV_sup, eV_sub)),
                               (256, 1, (eU_sup, eU_sub)),
                               (384, 1, (eV_sup, eV_sub))):
        if et != 0.0:
            hk_np[2 * rb, col0] = et
        if eb != 0.0:
            hk_np[2 * rb + 1, col0 + 127] = eb

    # Padded slab: vpad[i, j] = u[i-1, j-1]; boundary fills per the reference.
    irho = np.float32(1.0 / rho)
    vpad = np.empty((NX + 2, NY + 2), dtype=np.float32)
    vpad[1:-1, 1:-1] = u
    vpad[0, :] = bc0
    vpad[-1, :] = bc1
    vpad[:, 0] = bc0
    vpad[:, -1] = bc1
    vpad *= irho
    vpad = vpad.astype(np.float16)

    in_maps = []
    for k in range(M):
        r0 = k * RPC
        slab = np.ascontiguousarray(vpad[r0 : r0 + RPC + 2, :])
        # halo rows: {rb0 top, rb0 bottom, rb1 top, rb1 bottom}
        hxm = slab[[0, P + 1, P, RPC + 1], :]
        im = {"v": slab, "hk": hk_np}
        for i in range(4):
            im[f"hx{i}"] = np.ascontiguousarray(hxm[:, i * CH : i * CH + 514])
        in_maps.append(im)

    res = run_bass_kernel_spmd(nc, in_maps, core_ids=list(range(M)))
    full = np.empty((NX, NY), dtype=np.float32)
    for k in range(M):
        rres = res.results[k]
        row0 = k * RPC
        for rb in range(NRB):
            for h in range(2):
                full[row0 + rb * P : row0 + (rb + 1) * P,
                     h * HW : (h + 1) * HW] = rres[f"o{rb}{h}"]
    return full


# revision 41
# speedup vs baseline: 1.0111x; 1.0008x over previous
"""Trainium2 Bass kernel for FINN-Burger2D flux step (2048x2048, 8 NeuronCores).

Strategy (v7 - f16 pipeline, chunk-granular schedule; sim/HW 15977 ns)
----------------------------------------------------------------------
The per-point MLP a(u) = W3^T tanh(W2^T tanh(W1^T u)) is approximated by
a(u) ~= c*tanh(al*u) + cL*u (re-fit at runtime; the diffusion term d*S is
absorbed into the fit target).  With n2 = a/cL and kappa = |cL|/(2*DX):

    out = n2 * W,   W = kappa*(S + sig*T)   if u > 0  (branch "U")
                    W = kappa*(-S + sig*T)  otherwise (branch "V")

Each branch is a 3-point stencil computed by banded matmuls into PSUM
(band diag+offdiag, column-shift diag, K=4 halo = 3 matmuls per chunk per
branch), then a copy_predicated select and a final n2-multiply.

Schedule/format choices (vs the 18720 ns v3 baseline):
 - Inputs travel as float16, pre-scaled by 1/rho host-side (rho**2 folded
   into the stencil lhsT constants, rho into the tanh input scale), so
   n2 = t1 + u' is a plain tensor_tensor add, loads cost half the DMA
   time, and f16 matmuls have no sub-256-column penalty.
 - No PE warmup: the cost model ramps the PE clock on wall-clock time,
   so real matmuls start as soon as the first 500ns load slice lands
   (~2.2us) and run mid-clock until t=3us.
 - 12 chunks pipelined at 128-512 column granularity; every chunk's
   select/multiply/store runs right behind its matmuls.  The last unit is
   [256,256,256,128,128] with the two 128-col chunks' outputs merged into
   one store, minimizing the post-PE tail (pred+mult+store+DMA-init).
 - Engine map: tanh + PSUM->f16 staging on ACT; masks, n2 adds, selects,
   and tail multiplies on DVE; staged multiplies + the unit-3 n2 (pinned
   at 9.2us) on Pool; lhsT band/diag blocks generated on-chip on Pool
   (interleaved chains hide semaphore gaps), halo lhsT via a DMA'd
   constant; halo-row loads pinned into queue idle windows with
   tile_wait_until; stores placed per-queue so the final chains
   (completion = end + 1717ns HWDGE / 1883ns SWDGE init) balance.
"""

import numpy as np

import concourse.bass as bass
import concourse.mybir as mybir
import concourse.tile as tile
from concourse.bacc import Bacc
from concourse.bass_utils import run_bass_kernel_spmd
from concourse.vector_clock import ScopedClock, VectorClock


def _chunked_drain_and_barrier(self, tick_clock, wait_clock):
    """Tail drain split into <=1-wait chunks (walrus rejects ~11 waits on one
    instruction: 'Too many sync wait commands')."""
    gc = tick_clock.global_clock
    full = list(gc)
    procs = [i for i, t in enumerate(full) if t > 0]
    CHUNK = 4
    for i in range(0, len(procs), CHUNK):
        sub = [0] * len(full)
        for p in procs[i : i + CHUNK]:
            sub[p] = full[p]
        d = self.nc.sync.drain()
        wait_clock.add_sem_waits(d.ins, ScopedClock({None: VectorClock(sub)}))
    self.nc.sync.drain()

    self.nc.all_engine_barrier()
    assert self.sems is not None
    popped = self.nc._tile_sem_poison_stack.pop()
    assert popped is self._sem_poison
    self.nc.clear_and_free_semaphores(list(self.sems.allocated().values()))
    self.nc.all_engine_barrier()


tile.TileContext._drain_and_barrier = _chunked_drain_and_barrier

F32 = mybir.dt.float32
F32R = mybir.dt.float32r
F16 = mybir.dt.float16
I16 = mybir.dt.int16
BF16 = mybir.dt.bfloat16
AF = mybir.ActivationFunctionType
ALU = mybir.AluOpType

NX = 2048
NY = 2048
DX = 0.01
M = 8                 # cores
RPC = NX // M         # 256 rows per core
P = 128               # partitions
NRB = RPC // P        # row blocks per core (2)
CH = 512              # matmul free-dim chunk (one fp32 PSUM bank)
HW = NY // 2          # half width (1024)

# Starting alpha for the runtime fit (solved offline for the seed-0 weights).
FIT_ALPHA = 1.25307


def _mlp_scalar(x, W1, W2, W3):
    h = np.tanh(x[:, None] * W1[0])
    h = np.tanh(h @ W2)
    return (h @ W3)[:, 0]


def _fit_units(W1, W2, W3, d):
    """Fit a(u) - 2*DX*d*sgn(u) ~= c*tanh(al*u) + cL*u on u>0.

    The -2*DX*d shift absorbs the diffusion term d*S into |a|/(2DX)*S
    exactly; the T-term picks up a d*T-sized error (~2e-4 relative).
    Lawson-weighted lstsq for the minimax coefficients; scipy LM polish of
    alpha when the hardcoded start is stale.
    """
    xs = np.linspace(1e-4, 5.7, 4001)
    fx = _mlp_scalar(xs, W1, W2, W3) - 2.0 * DX * d

    def basis(al):
        return np.stack([np.tanh(al * xs), xs], axis=1)

    def lawson(al, iters=80):
        w = np.ones_like(xs)
        best_m, best_c = np.inf, None
        for _ in range(iters):
            A = basis(al) * w[:, None]
            c, *_ = np.linalg.lstsq(A, fx * w, rcond=None)
            r = basis(al) @ c - fx
            m = float(np.abs(r).max())
            if m < best_m:
                best_m, best_c = m, c.copy()
            w *= np.sqrt(np.abs(r) + 1e-14)
            w /= w.max()
        return best_m, best_c

    al = float(FIT_ALPHA)
    m, c = lawson(al)
    if m > 4.0e-3:
        try:
            from scipy.optimize import least_squares

            def cost(la):
                A = basis(float(np.exp(la[0])))
                cc, *_ = np.linalg.lstsq(A, fx, rcond=None)
                return A @ cc - fx

            sol = least_squares(cost, [np.log(al)], method="lm", max_nfev=400)
            al2 = float(np.exp(sol.x[0]))
            m2, c2 = lawson(al2)
            if m2 < m:
                al, m, c = al2, m2, c2
        except Exception:
            pass
    return al, c, m


_CACHE = {}
_TRACE_SIM = False
_LAST_TC = [None]
NDUMMY = 0
PIN_HH1 = 2.0e-3
PIN_HH2 = 3.0e-3
PIN_HH3 = 4.0e-3
PIN_N23 = 8.6e-3


def _build_program(al, sig, kap, s0, s1):
    """Emit the per-core Bass program.

    al: tanh input scale; rho = c/cL (STT combine ratio); sig = sgn(cL);
    kap = |cL|/(2*DX) folded into the stencil constants.
    """
    nc = Bacc()
    v = nc.dram_tensor("v", [RPC + 2, NY + 2], F16, kind="ExternalInput")
    # Halo rows {rb0 top, rb0 bottom, rb1 top, rb1 bottom} in four
    # per-512-chunk tensors: DMA queue cost scales with free-dim bytes only.
    hxs = [nc.dram_tensor(f"hx{i}", [4, 514], F16, kind="ExternalInput")
           for i in range(4)]
    # halo lhsT constants, prebuilt host-side:
    # hk 4 blocks of [4,128]: rb0-U, rb0-V, rb1-U, rb1-V.
    hkd = nc.dram_tensor("hk", [4, 512], F16, kind="ExternalInput")
    outs = [[nc.dram_tensor(f"o{rb}{h}", [P, HW], F32, kind="ExternalOutput")
             for h in range(2)] for rb in range(NRB)]

    # lhsT coefficients.  U branch taken where n2 > 0 (<=> u > 0).
    eU_diag = 4.0 * kap * s0
    eU_sup = kap * s1 * (1.0 + sig)     # u[r-1] coeff, lhsT[k, k+1]
    eU_sub = kap * s1 * (1.0 - sig)     # u[r+1] coeff, lhsT[k, k-1]
    eV_diag = -4.0 * kap * s0
    eV_sup = kap * s1 * (sig - 1.0)
    eV_sub = kap * s1 * (-1.0 - sig)

    tc_obj = tile.TileContext(nc, trace_sim=_TRACE_SIM)
    with tc_obj as tc:
        with (
            tc.tile_pool(name="cg", bufs=1) as cg,
            tc.tile_pool(name="io", bufs=1) as io,
            tc.tile_pool(name="wk", bufs=1) as wk,
            tc.tile_pool(name="oo", bufs=1) as oo,
            tc.tile_pool(name="ps", bufs=1, space="PSUM") as ps,
        ):
            # ---- on-chip lhsT generation for the band/diag blocks (Pool,
            # ~1us: ready before the first matmul at ~2.3us; a DMA'd
            # constant would not be, due to the ~1.9us SWDGE init).
            # cpack cols: [0:128]=bandU [128:256]=bandV [256:384]=diagU
            # [384:512]=diagV.
            cpackf = cg.tile([P, 512], F32)
            cpack = cg.tile([P, 512], F16)
            AFF = [[-1, 128]]

            def gen_band(tmp, tmp2, col0, ediag, esup, esub):
                nc.gpsimd.memset(tmp[:], float(ediag))
                nc.gpsimd.affine_select(cpackf[:, col0 : col0 + 128], tmp[:],
                                        AFF, ALU.is_equal, 0.0, base=0,
                                        channel_multiplier=1)
                eoff, boff = (esup, 1) if esup != 0.0 else (esub, -1)
                if eoff != 0.0:
                    # lhsT[k, k+1] => p - f == -1 => base=+1 makes it ==0
                    nc.gpsimd.memset(tmp[:], float(eoff))
                    nc.gpsimd.affine_select(tmp2[:], tmp[:], AFF, ALU.is_equal,
                                            0.0, base=boff, channel_multiplier=1)
                    nc.gpsimd.tensor_tensor(cpackf[:, col0 : col0 + 128],
                                            cpackf[:, col0 : col0 + 128],
                                            tmp2[:], ALU.add)
                nc.gpsimd.tensor_copy(cpack[:, col0 : col0 + 128],
                                      cpackf[:, col0 : col0 + 128])

            def gen_diag(tmp, col0, coef):
                nc.gpsimd.memset(tmp[:], float(coef))
                nc.gpsimd.affine_select(cpackf[:, col0 : col0 + 128], tmp[:],
                                        AFF, ALU.is_equal, 0.0, base=0,
                                        channel_multiplier=1)
                nc.gpsimd.tensor_copy(cpack[:, col0 : col0 + 128],
                                      cpackf[:, col0 : col0 + 128])

            # Pool head: warm-source memset, hh0 DMA (ready ~3.0us,
            # chunk0 halo matmuls ~3.2us), then the gen chain with
            # per-block tmp tiles so the four chains interleave on Pool and
            # hide each other's semaphore gaps.  hk rides the ACT queue
            # (ready ~2.6us).  wait_until keeps the later halo loads from
            # occupying Pool inside the gen chain's semaphore gaps.
            wsc16 = cg.tile([1, 16], F32)
            nc.gpsimd.memset(wsc16[:], 0.25)
            hh0 = io.tile([4, 514], F16, tag="hh0")
            nc.gpsimd.dma_start(hh0[:], hxs[0][:, :])
            hk = cg.tile([4, 512], F16)
            nc.scalar.dma_start(hk[:], hkd[:, :])
            tmpVa = cg.tile([P, 128], F32)
            tmpVb = cg.tile([P, 128], F32)
            tmpUa = cg.tile([P, 128], F32)
            tmpUb = cg.tile([P, 128], F32)
            tmpD1 = cg.tile([P, 128], F32)
            tmpD2 = cg.tile([P, 128], F32)
            gen_band(tmpVa, tmpVb, 128, eV_diag, eV_sup, eV_sub)
            gen_band(tmpUa, tmpUb, 0, eU_diag, eU_sup, eU_sub)
            gen_diag(tmpD1, 384, eV_sup if eV_sup != 0.0 else eV_sub)
            gen_diag(tmpD2, 256, eU_sup if eU_sup != 0.0 else eU_sub)

            hh1 = io.tile([4, 514], F16, tag="hh1")
            hh2 = io.tile([4, 514], F16, tag="hh2")
            hh3 = io.tile([4, 514], F16, tag="hh3")
            with tc.tile_wait_until(PIN_HH1):
                nc.gpsimd.dma_start(hh1[:], hxs[1][:, :])
            with tc.tile_wait_until(PIN_HH2):
                nc.gpsimd.dma_start(hh2[:], hxs[2][:, :])
            with tc.tile_wait_until(PIN_HH3):
                nc.gpsimd.dma_start(hh3[:], hxs[3][:, :])
            hhs = [hh0, hh1, hh2, hh3]

            # ACT table warm: the first real Tanh would otherwise pay the
            # ~1.3us activation-table load.
            warm = cg.tile([1, 16], F16)
            nc.scalar.activation(warm[:], wsc16[:], AF.Tanh, scale=1.0)

            # ---- slab loads ----
            # ucA0 sliced so chunk0 (256 cols) computes at ~2.3us off the
            # first 500ns slice.
            HW2 = HW + 2
            uc = [[None, None], [None, None]]
            ucA0 = io.tile([P, HW2], F16, tag="ucA0")
            nc.sync.dma_start(ucA0[:, 0:258], v[1 : P + 1, 0:258])
            nc.sync.dma_start(ucA0[:, 258:HW2], v[1 : P + 1, 258:HW2])
            uc[0][0] = ucA0
            ucB0 = io.tile([P, HW2], F16, tag="ucB0")
            nc.sync.dma_start(ucB0[:], v[1 : P + 1, HW : NY + 2])
            uc[0][1] = ucB0
            ucA1 = io.tile([P, HW2], F16, tag="ucA1")
            nc.sync.dma_start(ucA1[:], v[P + 1 : RPC + 1, 0:HW2])
            uc[1][0] = ucA1
            ucB1 = io.tile([P, HW2], F16, tag="ucB1")
            nc.sync.dma_start(ucB1[:], v[P + 1 : RPC + 1, HW : NY + 2])
            uc[1][1] = ucB1

            mop = ALU.min if sig > 0 else ALU.max

            # per-unit elementwise tiles
            masks, n2s = {}, {}

            def unit_elementwise(rb, h):
                ut = uc[rb][h]
                center = ut
                t1 = wk.tile([P, HW], F16, tag=f"t1{rb}{h}")
                mask = wk.tile([P, HW], F16, tag=f"mask{rb}{h}")
                n2 = wk.tile([P, HW], F16, tag=f"n2{rb}{h}")
                # unit0 follows the sliced load; hh3 DMA slotted after the
                # first tanh piece.
                acts = ([slice(0, 256), slice(256, CH), slice(CH, HW)]
                        if (rb == 0 and h == 0) else [slice(0, HW)])
                for k, cs in enumerate(acts):
                    ctr = center[:, 1 + cs.start : 1 + cs.stop]
                    nc.scalar.activation(t1[:, cs], ctr, AF.Tanh, scale=float(al))
                    nc.vector.tensor_scalar(mask[:, cs], t1[:, cs], 0.0, None, mop)
                    # u arrives pre-scaled by 1/rho (rho**2 folded into the
                    # stencil constants), so n2' = t1 + u' is a plain add --
                    # the fused scalar_tensor_tensor is not a legal Pool op
                    # on hardware.
                    if rb == 1 and h == 1:
                        # off the DVE tail-pred cascade; pinned into Pool's
                        # idle window
                        with tc.tile_wait_until(PIN_N23):
                            nc.gpsimd.tensor_tensor(n2[:, cs], t1[:, cs], ctr,
                                                    ALU.add)
                    else:
                        nc.vector.tensor_tensor(n2[:, cs], t1[:, cs], ctr,
                                                ALU.add)
                masks[(rb, h)] = mask
                n2s[(rb, h)] = n2

            # chunk list: (rb, h, col0-in-unit, width).  First unit split
            # [256,256,512] so chunk0 needs only the first load slice; last
            # unit [512,256,256] so the post-PE tail chain is small.
            chunks = [(0, 0, 0, 256), (0, 0, 256, 256), (0, 0, 512, CH),
                      (0, 1, 0, CH), (0, 1, CH, CH),
                      (1, 0, 0, CH), (1, 0, CH, CH),
                      (1, 1, 0, 256), (1, 1, 256, 256), (1, 1, 512, 256),
                      (1, 1, 768, 128), (1, 1, 896, 128)]

            # mult plan: 'd' = DVE direct from PSUM; 'p' = ACT f16 stage +
            # Pool multiply.  Store queues spread over SP/ACT/Pool.
            MULT_ENG = ['d', 'd', 'p', 'p', 'p', 'p', 'p', 'p', 'p', 'p',
                        'd', 'd']
            # c7+c8 and c10+c11 write shared ot tiles, stored by one DMA
            # each (fewer 500ns descriptor floors in the tail window)
            STORE_ENG = [nc.sync, nc.sync, nc.sync, nc.sync, nc.sync,
                         nc.gpsimd, nc.sync, None, nc.scalar, nc.gpsimd,
                         None, nc.sync]
            # emit each unit's elementwise one chunk ahead of its first use
            EMIT_UNIT = {0: (0, 0), 2: (0, 1), 4: (1, 0), 6: (1, 1)}

            for ci, (rb, h, c0, w) in enumerate(chunks):
                if ci in EMIT_UNIT:
                    unit_elementwise(*EMIT_UNIT[ci])
                ut = uc[rb][h]
                c0g = h * HW + c0               # global col in row
                cs = slice(c0, c0 + w)
                rc = ut[:, c0 + 1 : c0 + w + 1]
                rm = ut[:, c0 : c0 + w]
                rp = ut[:, c0 + 2 : c0 + w + 2]
                rhsU = rm if eU_sup != 0.0 else rp
                rhsV = rm if eV_sup != 0.0 else rp
                hh = hhs[c0g // CH]
                hb = 1 + c0g - (c0g // CH) * CH
                rh = hh[0:4, hb : hb + w]
                hU = hk[0:4, 256 * rb : 256 * rb + 128]
                hV = hk[0:4, 256 * rb + 128 : 256 * rb + 256]

                psU = ps.tile([P, CH], F32, tag=f"U{ci % 4}")
                psV = ps.tile([P, CH], F32, tag=f"V{ci % 4}")
                pu = psU[:, 0:w]
                pv = psV[:, 0:w]
                nc.tensor.matmul(pv, cpack[:, 128:256], rc, start=True, stop=False)
                nc.tensor.matmul(pv, cpack[:, 384:512], rhsV, start=False, stop=False)
                nc.tensor.matmul(pu, cpack[:, 0:128], rc, start=True, stop=False)
                nc.tensor.matmul(pu, cpack[:, 256:384], rhsU, start=False, stop=False)
                nc.tensor.matmul(pv, hV, rh, start=False, stop=True)
                nc.tensor.matmul(pu, hU, rh, start=False, stop=True)

                mask = masks[(rb, h)]
                n2 = n2s[(rb, h)]
                nc.vector.copy_predicated(pv, mask[:, cs].bitcast(I16), pu)
                if ci in (7, 8):
                    if ci == 7:
                        share78 = oo.tile([P, CH], F32, tag="ot78",
                                          name="ot78")
                    ob = (ci - 7) * 256
                    ot = share78
                elif ci in (10, 11):
                    if ci == 10:
                        share1011 = oo.tile([P, 256], F32, tag="ot1011",
                                            name="ot1011")
                    ob = (ci - 10) * 128
                    ot = share1011
                else:
                    ot = oo.tile([P, CH], F32, tag=f"ot{ci}")
                    ob = 0
                if MULT_ENG[ci] == 'd':
                    nc.vector.tensor_mul(ot[:, ob : ob + w], n2[:, cs], pv)
                else:
                    wsb = wk.tile([P, CH], F16, tag=f"wsb{ci}")
                    nc.scalar.activation(wsb[:, 0:w], pv, AF.Copy, scale=1.0)
                    nc.gpsimd.tensor_mul(ot[:, ob : ob + w], n2[:, cs],
                                         wsb[:, 0:w])
                if ci == 8:
                    STORE_ENG[ci].dma_start(outs[rb][h][:, 0:512], ot[:, 0:512])
                elif ci == 11:
                    STORE_ENG[ci].dma_start(outs[rb][h][:, 768:1024],
                                            ot[:, 0:256])
                elif STORE_ENG[ci] is not None:
                    STORE_ENG[ci].dma_start(outs[rb][h][:, cs], ot[:, 0:w])
    _LAST_TC[0] = tc_obj
    nc.finalize()
    return nc


def kernel(u, W1, W2, W3, D, BC, stencil):
    u = np.ascontiguousarray(u, dtype=np.float32)
    W1 = np.asarray(W1, dtype=np.float32)
    W2 = np.asarray(W2, dtype=np.float32)
    W3 = np.asarray(W3, dtype=np.float32)
    d = float(np.asarray(D).ravel()[0])
    bc0 = float(np.asarray(BC)[0, 0])
    bc1 = float(np.asarray(BC)[1, 0])
    s0 = float(np.asarray(stencil)[0])
    s1 = float(np.asarray(stencil)[1])

    al, cc, _ = _fit_units(W1, W2, W3, d)
    rho = cc[0] / cc[1]
    sig = 1.0 if cc[1] >= 0 else -1.0
    kap = abs(cc[1]) / (2.0 * DX)
    # the device program sees u' = u/rho, tanh scale al*rho, and stencil
    # constants kap*rho**2, making n2' = t1 + u' a plain add (out is
    # n2'*W' = n2*W exactly)
    al_dev = al * rho
    kap_dev = kap * rho * rho

    key = (round(al_dev, 10), sig,
           round(kap_dev, 8), round(s0, 10), round(s1, 10))
    if key not in _CACHE:
        _CACHE.clear()
        _CACHE[key] = _build_program(al_dev, sig, kap_dev, s0, s1)
    nc = _CACHE[key]

    # lhsT constant blocks (layout documented in _build_program)
    eU_sup = kap_dev * s1 * (1.0 + sig)
    eU_sub = kap_dev * s1 * (1.0 - sig)
    eV_sup = kap_dev * s1 * (sig - 1.0)
    eV_sub = kap_dev * s1 * (-1.0 - sig)
    hk_np = np.zeros((4, 512), dtype=np.float16)
    for col0, rb, (et, eb) in ((0, 0, (eU_sup, eU_sub)),
                               (128, 0, (eV_sup, eV_sub)),
                               (256, 1, (eU_sup, eU_sub)),
                               (384, 1, (eV_sup, eV_sub))):
        if et != 0.0:
            hk_np[2 * rb, col0] = et
        if eb != 0.0:
            hk_np[2 * rb + 1, col0 + 127] = eb

    # Padded slab: vpad[i, j] = u[i-1, j-1]; boundary fills per the reference.
    irho = np.float32(1.0 / rho)
    vpad = np.empty((NX + 2, NY + 2), dtype=np.float32)
    vpad[1:-1, 1:-1] = u
    vpad[0, :] = bc0
    vpad[-1, :] = bc1
    vpad[:, 0] = bc0
    vpad[:, -1] = bc1
    vpad *= irho
    vpad = vpad.astype(np.float16)

    in_maps = []
    for k in range(M):
        r0 = k * RPC
        slab = np.ascontiguousarray(vpad[r0 : r0 + RPC + 2, :])
        # halo rows: {rb0 top, rb0 bottom, rb1 top, rb1 bottom}
        hxm = slab[[0, P + 1, P, RPC + 1], :]
        im = {"v": slab, "hk": hk_np}
        for i in range(4):
            im[f"hx{i}"] = np.ascontiguousarray(hxm[:, i * CH : i * CH + 514])
        in_maps.append(im)

    res = run_bass_kernel_spmd(nc, in_maps, core_ids=list(range(M)))
    full = np.empty((NX, NY), dtype=np.float32)
    for k in range(M):
        rres = res.results[k]
        row0 = k * RPC
        for rb in range(NRB):
            for h in range(2):
                full[row0 + rb * P : row0 + (rb + 1) * P,
                     h * HW : (h + 1) * HW] = rres[f"o{rb}{h}"]
    return full


# revision 42
# speedup vs baseline: 1.0115x; 1.0004x over previous
"""Trainium2 Bass kernel for FINN-Burger2D flux step (2048x2048, 8 NeuronCores).

Strategy (v7 - f16 pipeline, chunk-granular schedule; sim/HW 15977 ns)
----------------------------------------------------------------------
The per-point MLP a(u) = W3^T tanh(W2^T tanh(W1^T u)) is approximated by
a(u) ~= c*tanh(al*u) + cL*u (re-fit at runtime; the diffusion term d*S is
absorbed into the fit target).  With n2 = a/cL and kappa = |cL|/(2*DX):

    out = n2 * W,   W = kappa*(S + sig*T)   if u > 0  (branch "U")
                    W = kappa*(-S + sig*T)  otherwise (branch "V")

Each branch is a 3-point stencil computed by banded matmuls into PSUM
(band diag+offdiag, column-shift diag, K=4 halo = 3 matmuls per chunk per
branch), then a copy_predicated select and a final n2-multiply.

Schedule/format choices (vs the 18720 ns v3 baseline):
 - Inputs travel as float16, pre-scaled by 1/rho host-side (rho**2 folded
   into the stencil lhsT constants, rho into the tanh input scale), so
   n2 = t1 + u' is a plain tensor_tensor add, loads cost half the DMA
   time, and f16 matmuls have no sub-256-column penalty.
 - No PE warmup: the cost model ramps the PE clock on wall-clock time,
   so real matmuls start as soon as the first 500ns load slice lands
   (~2.2us) and run mid-clock until t=3us.
 - 12 chunks pipelined at 128-512 column granularity; every chunk's
   select/multiply/store runs right behind its matmuls.  The last unit is
   [256,256,256,128,128] with the two 128-col chunks' outputs merged into
   one store, minimizing the post-PE tail (pred+mult+store+DMA-init).
 - Engine map: tanh + PSUM->f16 staging on ACT; masks, n2 adds, selects,
   and tail multiplies on DVE; staged multiplies + the unit-3 n2 (pinned
   at 9.2us) on Pool; lhsT band/diag blocks generated on-chip on Pool
   (interleaved chains hide semaphore gaps), halo lhsT via a DMA'd
   constant; halo-row loads pinned into queue idle windows with
   tile_wait_until; stores placed per-queue so the final chains
   (completion = end + 1717ns HWDGE / 1883ns SWDGE init) balance.
"""

import numpy as np

import concourse.bass as bass
import concourse.mybir as mybir
import concourse.tile as tile
from concourse.bacc import Bacc
from concourse.bass_utils import run_bass_kernel_spmd
from concourse.vector_clock import ScopedClock, VectorClock


def _chunked_drain_and_barrier(self, tick_clock, wait_clock):
    """Tail drain split into <=1-wait chunks (walrus rejects ~11 waits on one
    instruction: 'Too many sync wait commands')."""
    gc = tick_clock.global_clock
    full = list(gc)
    procs = [i for i, t in enumerate(full) if t > 0]
    CHUNK = 4
    for i in range(0, len(procs), CHUNK):
        sub = [0] * len(full)
        for p in procs[i : i + CHUNK]:
            sub[p] = full[p]
        d = self.nc.sync.drain()
        wait_clock.add_sem_waits(d.ins, ScopedClock({None: VectorClock(sub)}))
    self.nc.sync.drain()

    self.nc.all_engine_barrier()
    assert self.sems is not None
    popped = self.nc._tile_sem_poison_stack.pop()
    assert popped is self._sem_poison
    self.nc.clear_and_free_semaphores(list(self.sems.allocated().values()))
    self.nc.all_engine_barrier()


tile.TileContext._drain_and_barrier = _chunked_drain_and_barrier

F32 = mybir.dt.float32
F32R = mybir.dt.float32r
F16 = mybir.dt.float16
I16 = mybir.dt.int16
BF16 = mybir.dt.bfloat16
AF = mybir.ActivationFunctionType
ALU = mybir.AluOpType

NX = 2048
NY = 2048
DX = 0.01
M = 8                 # cores
RPC = NX // M         # 256 rows per core
P = 128               # partitions
NRB = RPC // P        # row blocks per core (2)
CH = 512              # matmul free-dim chunk (one fp32 PSUM bank)
HW = NY // 2          # half width (1024)

# Starting alpha for the runtime fit (solved offline for the seed-0 weights).
FIT_ALPHA = 1.25307


def _mlp_scalar(x, W1, W2, W3):
    h = np.tanh(x[:, None] * W1[0])
    h = np.tanh(h @ W2)
    return (h @ W3)[:, 0]


def _fit_units(W1, W2, W3, d):
    """Fit a(u) - 2*DX*d*sgn(u) ~= c*tanh(al*u) + cL*u on u>0.

    The -2*DX*d shift absorbs the diffusion term d*S into |a|/(2DX)*S
    exactly; the T-term picks up a d*T-sized error (~2e-4 relative).
    Lawson-weighted lstsq for the minimax coefficients; scipy LM polish of
    alpha when the hardcoded start is stale.
    """
    xs = np.linspace(1e-4, 5.7, 4001)
    fx = _mlp_scalar(xs, W1, W2, W3) - 2.0 * DX * d

    def basis(al):
        return np.stack([np.tanh(al * xs), xs], axis=1)

    def lawson(al, iters=80):
        w = np.ones_like(xs)
        best_m, best_c = np.inf, None
        for _ in range(iters):
            A = basis(al) * w[:, None]
            c, *_ = np.linalg.lstsq(A, fx * w, rcond=None)
            r = basis(al) @ c - fx
            m = float(np.abs(r).max())
            if m < best_m:
                best_m, best_c = m, c.copy()
            w *= np.sqrt(np.abs(r) + 1e-14)
            w /= w.max()
        return best_m, best_c

    al = float(FIT_ALPHA)
    m, c = lawson(al)
    if m > 4.0e-3:
        try:
            from scipy.optimize import least_squares

            def cost(la):
                A = basis(float(np.exp(la[0])))
                cc, *_ = np.linalg.lstsq(A, fx, rcond=None)
                return A @ cc - fx

            sol = least_squares(cost, [np.log(al)], method="lm", max_nfev=400)
            al2 = float(np.exp(sol.x[0]))
            m2, c2 = lawson(al2)
            if m2 < m:
                al, m, c = al2, m2, c2
        except Exception:
            pass
    return al, c, m


_CACHE = {}
_TRACE_SIM = False
_LAST_TC = [None]
NDUMMY = 0
PIN_HH1 = 2.0e-3
PIN_HH2 = 2.9e-3
PIN_HH3 = 3.9e-3
PIN_N23 = 8.6e-3


def _build_program(al, sig, kap, s0, s1):
    """Emit the per-core Bass program.

    al: tanh input scale; rho = c/cL (STT combine ratio); sig = sgn(cL);
    kap = |cL|/(2*DX) folded into the stencil constants.
    """
    nc = Bacc()
    v = nc.dram_tensor("v", [RPC + 2, NY + 2], F16, kind="ExternalInput")
    # Halo rows {rb0 top, rb0 bottom, rb1 top, rb1 bottom} in four
    # per-512-chunk tensors: DMA queue cost scales with free-dim bytes only.
    hxs = [nc.dram_tensor(f"hx{i}", [4, 514], F16, kind="ExternalInput")
           for i in range(4)]
    # halo lhsT constants, prebuilt host-side:
    # hk 4 blocks of [4,128]: rb0-U, rb0-V, rb1-U, rb1-V.
    hkd = nc.dram_tensor("hk", [4, 512], F16, kind="ExternalInput")
    outs = [[nc.dram_tensor(f"o{rb}{h}", [P, HW], F32, kind="ExternalOutput")
             for h in range(2)] for rb in range(NRB)]

    # lhsT coefficients.  U branch taken where n2 > 0 (<=> u > 0).
    eU_diag = 4.0 * kap * s0
    eU_sup = kap * s1 * (1.0 + sig)     # u[r-1] coeff, lhsT[k, k+1]
    eU_sub = kap * s1 * (1.0 - sig)     # u[r+1] coeff, lhsT[k, k-1]
    eV_diag = -4.0 * kap * s0
    eV_sup = kap * s1 * (sig - 1.0)
    eV_sub = kap * s1 * (-1.0 - sig)

    tc_obj = tile.TileContext(nc, trace_sim=_TRACE_SIM)
    with tc_obj as tc:
        with (
            tc.tile_pool(name="cg", bufs=1) as cg,
            tc.tile_pool(name="io", bufs=1) as io,
            tc.tile_pool(name="wk", bufs=1) as wk,
            tc.tile_pool(name="oo", bufs=1) as oo,
            tc.tile_pool(name="ps", bufs=1, space="PSUM") as ps,
        ):
            # ---- on-chip lhsT generation for the band/diag blocks (Pool,
            # ~1us: ready before the first matmul at ~2.3us; a DMA'd
            # constant would not be, due to the ~1.9us SWDGE init).
            # cpack cols: [0:128]=bandU [128:256]=bandV [256:384]=diagU
            # [384:512]=diagV.
            cpackf = cg.tile([P, 512], F32)
            cpack = cg.tile([P, 512], F16)
            AFF = [[-1, 128]]

            def gen_band(tmp, tmp2, col0, ediag, esup, esub):
                nc.gpsimd.memset(tmp[:], float(ediag))
                nc.gpsimd.affine_select(cpackf[:, col0 : col0 + 128], tmp[:],
                                        AFF, ALU.is_equal, 0.0, base=0,
                                        channel_multiplier=1)
                eoff, boff = (esup, 1) if esup != 0.0 else (esub, -1)
                if eoff != 0.0:
                    # lhsT[k, k+1] => p - f == -1 => base=+1 makes it ==0
                    nc.gpsimd.memset(tmp[:], float(eoff))
                    nc.gpsimd.affine_select(tmp2[:], tmp[:], AFF, ALU.is_equal,
                                            0.0, base=boff, channel_multiplier=1)
                    nc.gpsimd.tensor_tensor(cpackf[:, col0 : col0 + 128],
                                            cpackf[:, col0 : col0 + 128],
                                            tmp2[:], ALU.add)
                nc.gpsimd.tensor_copy(cpack[:, col0 : col0 + 128],
                                      cpackf[:, col0 : col0 + 128])

            def gen_diag(tmp, col0, coef):
                nc.gpsimd.memset(tmp[:], float(coef))
                nc.gpsimd.affine_select(cpackf[:, col0 : col0 + 128], tmp[:],
                                        AFF, ALU.is_equal, 0.0, base=0,
                                        channel_multiplier=1)
                nc.gpsimd.tensor_copy(cpack[:, col0 : col0 + 128],
                                      cpackf[:, col0 : col0 + 128])

            # Pool head: warm-source memset, hh0 DMA (ready ~3.0us,
            # chunk0 halo matmuls ~3.2us), then the gen chain with
            # per-block tmp tiles so the four chains interleave on Pool and
            # hide each other's semaphore gaps.  hk rides the ACT queue
            # (ready ~2.6us).  wait_until keeps the later halo loads from
            # occupying Pool inside the gen chain's semaphore gaps.
            wsc16 = cg.tile([1, 16], F32)
            nc.gpsimd.memset(wsc16[:], 0.25)
            hh0 = io.tile([4, 514], F16, tag="hh0")
            nc.gpsimd.dma_start(hh0[:], hxs[0][:, :])
            hk = cg.tile([4, 512], F16)
            nc.scalar.dma_start(hk[:], hkd[:, :])
            tmpVa = cg.tile([P, 128], F32)
            tmpVb = cg.tile([P, 128], F32)
            tmpUa = cg.tile([P, 128], F32)
            tmpUb = cg.tile([P, 128], F32)
            tmpD1 = cg.tile([P, 128], F32)
            tmpD2 = cg.tile([P, 128], F32)
            gen_band(tmpVa, tmpVb, 128, eV_diag, eV_sup, eV_sub)
            gen_band(tmpUa, tmpUb, 0, eU_diag, eU_sup, eU_sub)
            gen_diag(tmpD1, 384, eV_sup if eV_sup != 0.0 else eV_sub)
            gen_diag(tmpD2, 256, eU_sup if eU_sup != 0.0 else eU_sub)

            hh1 = io.tile([4, 514], F16, tag="hh1")
            hh2 = io.tile([4, 514], F16, tag="hh2")
            hh3 = io.tile([4, 514], F16, tag="hh3")
            with tc.tile_wait_until(PIN_HH1):
                nc.gpsimd.dma_start(hh1[:], hxs[1][:, :])
            with tc.tile_wait_until(PIN_HH2):
                nc.gpsimd.dma_start(hh2[:], hxs[2][:, :])
            with tc.tile_wait_until(PIN_HH3):
                nc.gpsimd.dma_start(hh3[:], hxs[3][:, :])
            hhs = [hh0, hh1, hh2, hh3]

            # ACT table warm: the first real Tanh would otherwise pay the
            # ~1.3us activation-table load.
            warm = cg.tile([1, 16], F16)
            nc.scalar.activation(warm[:], wsc16[:], AF.Tanh, scale=1.0)

            # ---- slab loads ----
            # ucA0 sliced so chunk0 (256 cols) computes at ~2.3us off the
            # first 500ns slice.
            HW2 = HW + 2
            uc = [[None, None], [None, None]]
            ucA0 = io.tile([P, HW2], F16, tag="ucA0")
            nc.sync.dma_start(ucA0[:, 0:258], v[1 : P + 1, 0:258])
            nc.sync.dma_start(ucA0[:, 258:HW2], v[1 : P + 1, 258:HW2])
            uc[0][0] = ucA0
            ucB0 = io.tile([P, HW2], F16, tag="ucB0")
            nc.sync.dma_start(ucB0[:], v[1 : P + 1, HW : NY + 2])
            uc[0][1] = ucB0
            ucA1 = io.tile([P, HW2], F16, tag="ucA1")
            nc.sync.dma_start(ucA1[:], v[P + 1 : RPC + 1, 0:HW2])
            uc[1][0] = ucA1
            ucB1 = io.tile([P, HW2], F16, tag="ucB1")
            nc.sync.dma_start(ucB1[:], v[P + 1 : RPC + 1, HW : NY + 2])
            uc[1][1] = ucB1

            mop = ALU.min if sig > 0 else ALU.max

            # per-unit elementwise tiles
            masks, n2s = {}, {}

            def unit_elementwise(rb, h):
                ut = uc[rb][h]
                center = ut
                t1 = wk.tile([P, HW], F16, tag=f"t1{rb}{h}")
                mask = wk.tile([P, HW], F16, tag=f"mask{rb}{h}")
                n2 = wk.tile([P, HW], F16, tag=f"n2{rb}{h}")
                # unit0 follows the sliced load; hh3 DMA slotted after the
                # first tanh piece.
                acts = ([slice(0, 256), slice(256, CH), slice(CH, HW)]
                        if (rb == 0 and h == 0) else [slice(0, HW)])
                for k, cs in enumerate(acts):
                    ctr = center[:, 1 + cs.start : 1 + cs.stop]
                    nc.scalar.activation(t1[:, cs], ctr, AF.Tanh, scale=float(al))
                    nc.vector.tensor_scalar(mask[:, cs], t1[:, cs], 0.0, None, mop)
                    # u arrives pre-scaled by 1/rho (rho**2 folded into the
                    # stencil constants), so n2' = t1 + u' is a plain add --
                    # the fused scalar_tensor_tensor is not a legal Pool op
                    # on hardware.
                    if rb == 1 and h == 1:
                        # off the DVE tail-pred cascade; pinned into Pool's
                        # idle window
                        with tc.tile_wait_until(PIN_N23):
                            nc.gpsimd.tensor_tensor(n2[:, cs], t1[:, cs], ctr,
                                                    ALU.add)
                    else:
                        nc.vector.tensor_tensor(n2[:, cs], t1[:, cs], ctr,
                                                ALU.add)
                masks[(rb, h)] = mask
                n2s[(rb, h)] = n2

            # chunk list: (rb, h, col0-in-unit, width).  First unit split
            # [256,256,512] so chunk0 needs only the first load slice; last
            # unit [512,256,256] so the post-PE tail chain is small.
            chunks = [(0, 0, 0, 256), (0, 0, 256, 256), (0, 0, 512, CH),
                      (0, 1, 0, CH), (0, 1, CH, CH),
                      (1, 0, 0, CH), (1, 0, CH, CH),
                      (1, 1, 0, 256), (1, 1, 256, 256), (1, 1, 512, 256),
                      (1, 1, 768, 128), (1, 1, 896, 128)]

            # mult plan: 'd' = DVE direct from PSUM; 'p' = ACT f16 stage +
            # Pool multiply.  Store queues spread over SP/ACT/Pool.
            MULT_ENG = ['d', 'd', 'p', 'p', 'p', 'p', 'p', 'p', 'p', 'p',
                        'd', 'd']
            # c7+c8 and c10+c11 write shared ot tiles, stored by one DMA
            # each (fewer 500ns descriptor floors in the tail window)
            STORE_ENG = [nc.sync, nc.sync, nc.sync, nc.sync, nc.sync,
                         nc.gpsimd, nc.sync, None, nc.scalar, nc.gpsimd,
                         None, nc.sync]
            # emit each unit's elementwise one chunk ahead of its first use
            EMIT_UNIT = {0: (0, 0), 2: (0, 1), 4: (1, 0), 6: (1, 1)}

            for ci, (rb, h, c0, w) in enumerate(chunks):
                if ci in EMIT_UNIT:
                    unit_elementwise(*EMIT_UNIT[ci])
                ut = uc[rb][h]
                c0g = h * HW + c0               # global col in row
                cs = slice(c0, c0 + w)
                rc = ut[:, c0 + 1 : c0 + w + 1]
                rm = ut[:, c0 : c0 + w]
                rp = ut[:, c0 + 2 : c0 + w + 2]
                rhsU = rm if eU_sup != 0.0 else rp
                rhsV = rm if eV_sup != 0.0 else rp
                hh = hhs[c0g // CH]
                hb = 1 + c0g - (c0g // CH) * CH
                rh = hh[0:4, hb : hb + w]
                hU = hk[0:4, 256 * rb : 256 * rb + 128]
                hV = hk[0:4, 256 * rb + 128 : 256 * rb + 256]

                psU = ps.tile([P, CH], F32, tag=f"U{ci % 4}")
                psV = ps.tile([P, CH], F32, tag=f"V{ci % 4}")
                pu = psU[:, 0:w]
                pv = psV[:, 0:w]
                nc.tensor.matmul(pv, cpack[:, 128:256], rc, start=True, stop=False)
                nc.tensor.matmul(pv, cpack[:, 384:512], rhsV, start=False, stop=False)
                nc.tensor.matmul(pu, cpack[:, 0:128], rc, start=True, stop=False)
                nc.tensor.matmul(pu, cpack[:, 256:384], rhsU, start=False, stop=False)
                nc.tensor.matmul(pv, hV, rh, start=False, stop=True)
                nc.tensor.matmul(pu, hU, rh, start=False, stop=True)

                mask = masks[(rb, h)]
                n2 = n2s[(rb, h)]
                nc.vector.copy_predicated(pv, mask[:, cs].bitcast(I16), pu)
                if ci in (7, 8):
                    if ci == 7:
                        share78 = oo.tile([P, CH], F32, tag="ot78",
                                          name="ot78")
                    ob = (ci - 7) * 256
                    ot = share78
                elif ci in (10, 11):
                    if ci == 10:
                        share1011 = oo.tile([P, 256], F32, tag="ot1011",
                                            name="ot1011")
                    ob = (ci - 10) * 128
                    ot = share1011
                else:
                    ot = oo.tile([P, CH], F32, tag=f"ot{ci}")
                    ob = 0
                if MULT_ENG[ci] == 'd':
                    nc.vector.tensor_mul(ot[:, ob : ob + w], n2[:, cs], pv)
                else:
                    wsb = wk.tile([P, CH], F16, tag=f"wsb{ci}")
                    nc.scalar.activation(wsb[:, 0:w], pv, AF.Copy, scale=1.0)
                    nc.gpsimd.tensor_mul(ot[:, ob : ob + w], n2[:, cs],
                                         wsb[:, 0:w])
                if ci == 8:
                    STORE_ENG[ci].dma_start(outs[rb][h][:, 0:512], ot[:, 0:512])
                elif ci == 11:
                    STORE_ENG[ci].dma_start(outs[rb][h][:, 768:1024],
                                            ot[:, 0:256])
                elif STORE_ENG[ci] is not None:
                    STORE_ENG[ci].dma_start(outs[rb][h][:, cs], ot[:, 0:w])
    _LAST_TC[0] = tc_obj
    nc.finalize()
    return nc


def kernel(u, W1, W2, W3, D, BC, stencil):
    u = np.ascontiguousarray(u, dtype=np.float32)
    W1 = np.asarray(W1, dtype=np.float32)
    W2 = np.asarray(W2, dtype=np.float32)
    W3 = np.asarray(W3, dtype=np.float32)
    d = float(np.asarray(D).ravel()[0])
    bc0 = float(np.asarray(BC)[0, 0])
    bc1 = float(np.asarray(BC)[1, 0])
    s0 = float(np.asarray(stencil)[0])
    s1 = float(np.asarray(stencil)[1])

    al, cc, _ = _fit_units(W1, W2, W3, d)
    rho = cc[0] / cc[1]
    sig = 1.0 if cc[1] >= 0 else -1.0
    kap = abs(cc[1]) / (2.0 * DX)
    # the device program sees u' = u/rho, tanh scale al*rho, and stencil
    # constants kap*rho**2, making n2' = t1 + u' a plain add (out is
    # n2'*W' = n2*W exactly)
    al_dev = al * rho
    kap_dev = kap * rho * rho

    key = (round(al_dev, 10), sig,
           round(kap_dev, 8), round(s0, 10), round(s1, 10))
    if key not in _CACHE:
        _CACHE.clear()
        _CACHE[key] = _build_program(al_dev, sig, kap_dev, s0, s1)
    nc = _CACHE[key]

    # lhsT constant blocks (layout documented in _build_program)
    eU_sup = kap_dev * s1 * (1.0 + sig)
    eU_sub = kap_dev * s1 * (1.0 - sig)
    eV_sup = kap_dev * s1 * (sig - 1.0)
    eV_sub = kap_dev * s1 * (-1.0 - sig)
    hk_np = np.zeros((4, 512), dtype=np.float16)
    for col0, rb, (et, eb) in ((0, 0, (eU_sup, eU_sub)),
                               (128, 0, (eV_sup, eV_sub)),
                               (256, 1, (eU_sup, eU_sub)),
                               (384, 1, (eV_sup, eV_sub))):
        if et != 0.0:
            hk_np[2 * rb, col0] = et
        if eb != 0.0:
            hk_np[2 * rb + 1, col0 + 127] = eb

    # Padded slab: vpad[i, j] = u[i-1, j-1]; boundary fills per the reference.
    irho = np.float32(1.0 / rho)
    vpad = np.empty((NX + 2, NY + 2), dtype=np.float32)
    vpad[1:-1, 1:-1] = u
    vpad[0, :] = bc0
    vpad[-1, :] = bc1
    vpad[:, 0] = bc0
    vpad[:, -1] = bc1
    vpad *= irho
    vpad = vpad.astype(np.float16)

    in_maps = []
    for k in range(M):
        r0 = k * RPC
        slab = np.ascontiguousarray(vpad[r0 : r0 + RPC + 2, :])
        # halo rows: {rb0 top, rb0 bottom, rb1 top, rb1 bottom}
        hxm = slab[[0, P + 1, P, RPC + 1], :]
        im = {"v": slab, "hk": hk_np}
        for i in range(4):
            im[f"hx{i}"] = np.ascontiguousarray(hxm[:, i * CH : i * CH + 514])
        in_maps.append(im)

    res = run_bass_kernel_spmd(nc, in_maps, core_ids=list(range(M)))
    full = np.empty((NX, NY), dtype=np.float32)
    for k in range(M):
        rres = res.results[k]
        row0 = k * RPC
        for rb in range(NRB):
            for h in range(2):
                full[row0 + rb * P : row0 + (rb + 1) * P,
                     h * HW : (h + 1) * HW] = rres[f"o{rb}{h}"]
    return full
